# revision 5
# baseline (speedup 1.0000x reference)
"""Local (windowed) attention with rotary embeddings on 8 TRN2 NeuronCores.

Problem: B=4 H=8 N=4096 D=64, window=128, look_backward=1 (j=256 keys/window),
rotary over position-in-context, causal+pad mask, softmax, PV.

Sharding: the packed (B*H)=32 batch axis is split across 8 cores, 4 rows each.
Windows are independent -> no cross-core communication.

Math notes (derived from reference.py, validated vs the jax reference):
  - Rotary phases depend only on position-in-context, identical for every
    window: q_i gets angle (128+i); key at context slot jj gets angle jj.
  - A key chunk (window c) appears in two contexts: slots [128,256) of window
    c (own) and slots [0,128) of window c+1 (prev).  Via R_a^T R_b = R_{b-a}
    we rotate k ONCE with angle jj' (0..127) and use two q rotations: qA with
    angle i (own chunk) and qB with angle i+128 (previous window's chunk).
  - Mask: own chunk causal (keep i >= jj'); prev chunk fully allowed; window
    0 has no prev chunk.
  - Logits are O(1) (scale folded into q-side rotary tables) so softmax skips
    max-subtraction; exp cannot overflow.

Dataflow (bf16 operands, fp32 accumulation; 2 windows per inner step):
  - DMA whole rows bf16: q,k [128, 32, 64]; vo [128, 32, 65] with a
    host-packed ones column (PV matmul then also emits the softmax denom).
  - Rotary products (cos-mul + sign-folded sin-mul via a rotate-half access
    pattern) on DVE/Pool in bf16 at 2x DVE throughput; the "+" of the two
    products happens for free in fp32 PSUM accumulation of the transposes.
  - Transposes run as REGULAR bf16 matmuls against a bf16 identity (1
    cycle/row vs 2 for fp32 is_transpose), accumulating cos+sin pairs into
    one fp32 PSUM bank: [qA^T (c,c+1) | qB^T (c+1,c+2) | k^T (c,c+1)].
  - One staging copy [128, 384] fp32->bf16 to SBUF per step.  Odd windows
    sit at partitions 64:128 and are consumed directly via tile_position
    (64, .) matmuls - mixing operand bases verified exact on this HW.
  - QK: one bf16 matmul per chunk, N=256 (rhs = [qA^T | qB^T+1] contiguous).
  - exp on ACT over [128, 1024] (4 windows per activation), causal mask as a
    DVE multiply with a constant 0/1 bf16 tile broadcast over own-blocks,
    PV in bf16 (fp32 accum), normalize = DVE reciprocal of the ones-column
    + one DVE multiply with the reciprocal broadcast per window.
"""

import numpy as np

import concourse.bass as bass
import concourse.bacc as bacc
import concourse.tile as tile
from concourse import mybir
from concourse.bass_utils import run_bass_kernel_spmd

B, H, N, D = 4, 8, 4096, 64
WIN = 128
NW = N // WIN            # 32 windows per row
NCORES = 8
ROWS = B * H             # 32 packed batch rows
RPC = ROWS // NCORES     # 4 rows per core
ROPE = 10000.0
SCALE = D ** -0.5

F32 = mybir.dt.float32
BF16 = mybir.dt.bfloat16

# staging engine for the PSUM->SBUF copy: "pool" if Pool can read PSUM on
# this toolchain, else "dve"
STAGE_ENGINE = "dve"
SEG = 16  # rotary mul segment (windows per DVE/Pool instruction)


def _rot_consts():
    """Host-side rotary constant tables, [WIN, D] each (float32 masters)."""
    inv = 1.0 / (ROPE ** (np.arange(0, D, 2, dtype=np.float64) / D))  # [D/2]

    def mats(t):
        fr = t[:, None] * inv[None, :]
        fr = np.concatenate([fr, fr], axis=-1)  # [WIN, D]
        return np.cos(fr), np.sin(fr)

    i = np.arange(WIN, dtype=np.float64)
    cosA, sinA = mats(i)          # q angle i        (vs own chunk, k angle jj')
    cosB, sinB = mats(i + WIN)    # q angle i+128    (vs prev chunk)
    cosK, sinK = mats(i)          # k angle jj'

    def fold_sin(s):
        # rotate_half contribution: out[:, :32] = in[:, 32:] * (-sin[:, :32])
        #                           out[:, 32:] = in[:, :32] * (+sin[:, 32:])
        f = s.copy()
        f[:, : D // 2] = -f[:, : D // 2]
        return f

    out = dict(
        cqA=cosA * SCALE, sqA=fold_sin(sinA) * SCALE,
        cqB=cosB * SCALE, sqB=fold_sin(sinB) * SCALE,
        cK=cosK, sK=fold_sin(sinK),
    )
    return {k: v.astype(np.float32) for k, v in out.items()}


CONST_NAMES = ["cqA", "sqA", "cqB", "sqB", "cK", "sK"]


def build_bass():
    nc = bacc.Bacc("TRN2", target_bir_lowering=False)
    # host pre-transposed layout [RPC, WIN, NW, D]: partition-major, so every
    # DMA moves multi-KB contiguous runs per partition
    q_d = nc.declare_dram_parameter("q", [RPC, WIN, NW, D], BF16, isOutput=False)
    k_d = nc.declare_dram_parameter("k", [RPC, WIN, NW, D], BF16, isOutput=False)
    vo_d = nc.declare_dram_parameter("vo", [RPC, WIN, NW, D + 1], BF16,
                                     isOutput=False)
    consts_d = {
        name: nc.declare_dram_parameter(name, [WIN, D], BF16, isOutput=False)
        for name in CONST_NAMES
    }
    ident_d = nc.declare_dram_parameter("ident", [WIN, WIN], BF16, isOutput=False)
    mask_d = nc.declare_dram_parameter("maskc", [WIN, WIN], BF16, isOutput=False)
    o_d = nc.declare_dram_parameter("o", [RPC, WIN, NW, D], BF16, isOutput=True)

    with tile.TileContext(nc) as tc:
        with (
            tc.tile_pool(name="singles", bufs=1) as singles,
            tc.tile_pool(name="rows", bufs=2) as rows,
            tc.tile_pool(name="rot", bufs=2) as rot,
            tc.tile_pool(name="stage", bufs=3) as s_pool,
            tc.tile_pool(name="win", bufs=2) as win_pool,
            tc.tile_pool(name="rec", bufs=2) as rec_pool,
            tc.tile_pool(name="tb", bufs=2, space="PSUM") as tb_pool,
            tc.tile_pool(name="psim", bufs=2, space="PSUM") as sim_pool,
            tc.tile_pool(name="po", bufs=2, space="PSUM") as po_pool,
        ):
            # ---- constants into SBUF
            c_sb = {}
            for name in CONST_NAMES:
                t = singles.tile([WIN, D], BF16, tag=f"const_{name}")
                nc.sync.dma_start(out=t, in_=consts_d[name][:, :])
                c_sb[name] = t
            ident_sb = singles.tile([WIN, WIN], BF16, tag="ident")
            nc.sync.dma_start(out=ident_sb, in_=ident_d[:, :])
            mask_sb = singles.tile([WIN, WIN], BF16, tag="maskc")
            nc.sync.dma_start(out=mask_sb, in_=mask_d[:, :])

            def bc(t, nwin):
                # [WIN, D] const -> broadcast over the window axis [WIN, nwin, D]
                return bass.AP(
                    tensor=t.tensor,
                    offset=t.offset,
                    ap=[list(t.ap[0]), [0, nwin], list(t.ap[1])],
                )

            def rot_view(t, w0, nwin):
                # rotate-half read: within each 64-block read [32:64] then [0:32]
                return bass.AP(
                    tensor=t.tensor,
                    offset=t.offset + w0 * D + 32,
                    ap=[list(t.ap[0]), [D, nwin], [-32, 2], [1, 32]],
                )

            stage_eng = nc.gpsimd if STAGE_ENGINE == "pool" else nc.vector

            for r in range(RPC):
                q_row = rows.tile([WIN, NW, D], BF16, tag="q_row")
                k_row = rows.tile([WIN, NW, D], BF16, tag="k_row")
                vo_row = rows.tile([WIN, NW, D + 1], BF16, tag="vo_row")
                out_row = rows.tile([WIN, NW, D], BF16, tag="out_row")
                nc.sync.dma_start(out=q_row, in_=q_d[r])
                nc.sync.dma_start(out=k_row, in_=k_d[r])
                nc.sync.dma_start(out=vo_row, in_=vo_d[r])

                # ---- rotary products (the "+" rides the PSUM accumulation)
                qcA = rot.tile([WIN, NW, D], BF16, tag="qcA")
                qsA = rot.tile([WIN, NW, D], BF16, tag="qsA")
                qcB = rot.tile([WIN, NW + 1, D], BF16, tag="qcB")
                qsB = rot.tile([WIN, NW + 1, D], BF16, tag="qsB")
                kc = rot.tile([WIN, NW, D], BF16, tag="kc")
                ks = rot.tile([WIN, NW, D], BF16, tag="ks")
                nc.vector.memset(qcB[:, NW, :], 0.0)
                nc.vector.memset(qsB[:, NW, :], 0.0)

                def rmul(eng, dst, src_row, cname, s0, half):
                    sl = slice(s0, s0 + SEG)
                    if half:  # sign-folded rotate-half product
                        eng.tensor_mul(
                            dst[:, sl, :].rearrange("p w (h d2) -> p w h d2", h=2),
                            rot_view(src_row, s0, SEG),
                            bc(c_sb[cname], SEG).rearrange(
                                "p w (h d2) -> p w h d2", h=2),
                        )
                    else:
                        eng.tensor_mul(dst[:, sl, :], src_row[:, sl, :],
                                       bc(c_sb[cname], SEG))

                for s0 in range(0, NW, SEG):
                    # all rotary muls on Pool: DVE is saturated by the
                    # PSUM-side work (staging/mask/recip) Pool cannot touch
                    rmul(nc.gpsimd, qcA, q_row, "cqA", s0, False)
                    rmul(nc.gpsimd, qsA, q_row, "sqA", s0, True)
                    rmul(nc.gpsimd, qcB, q_row, "cqB", s0, False)
                    rmul(nc.gpsimd, qsB, q_row, "sqB", s0, True)
                    rmul(nc.gpsimd, kc, k_row, "cK", s0, False)
                    rmul(nc.gpsimd, ks, k_row, "sK", s0, True)

                exp_prev = None
                sim4 = None
                exp4 = None
                for it in range(NW // 2):
                    c = 2 * it  # chunks (c, c+1); windows (c, c+1)
                    half = it % 2        # position inside the 4-window group
                    q0 = half * 4 * WIN

                    # ---- six transposes as regular bf16 matmuls vs identity,
                    # cos+sin accumulated in fp32 PSUM.  Packed layout:
                    # cols 0:128 qA^T (c, c+1) | 128:256 qB^T (c+1, c+2)
                    # | 256:384 k^T (c, c+1); second window of each pair lands
                    # on partitions 64:128.
                    TB = tb_pool.tile([WIN, 3 * WIN], F32)
                    for col, (ct, st, w0) in enumerate((
                        (qcA, qsA, c), (qcB, qsB, c + 1), (kc, ks, c),
                    )):
                        sl = TB[:, col * WIN : (col + 1) * WIN]
                        nc.tensor.matmul(sl, lhsT=ct[:, w0 : w0 + 2, :],
                                         rhs=ident_sb, start=True, stop=False)
                        nc.tensor.matmul(sl, lhsT=st[:, w0 : w0 + 2, :],
                                         rhs=ident_sb, start=False, stop=True)

                    # ---- one staging copy fp32 -> bf16 per step
                    S = s_pool.tile([WIN, 3 * WIN], BF16, tag="S")
                    stage_eng.tensor_copy(S, TB)

                    # ---- QK: one bf16 matmul per chunk, N=256
                    # sim4 cols (per 4-window group g, chunks 4g..4g+3):
                    #   [256*j : 256*j+128]     own logits of window 4g+j
                    #   [256*j+128 : 256*j+256] prev logits of window 4g+j+1
                    if half == 0:
                        sim4 = sim_pool.tile([WIN, 8 * WIN], F32)
                    nc.tensor.matmul(
                        sim4[:, q0 : q0 + 2 * WIN],
                        lhsT=S[0:64, 2 * WIN : 3 * WIN],
                        rhs=S[0:64, 0 : 2 * WIN],
                        start=True, stop=True,
                    )
                    # odd chunk reads partitions 64:128 of the same tile
                    nc.tensor.matmul(
                        sim4[:, q0 + 2 * WIN : q0 + 4 * WIN],
                        lhsT=S[64:128, 2 * WIN : 3 * WIN],
                        rhs=S[64:128, 0 : 2 * WIN],
                        start=True, stop=True,
                    )

                    if half == 0:
                        continue

                    # ---- exp + causal mask over the 4-window group
                    g0 = c - 2  # first window/chunk of the group
                    exp_prev = exp4
                    exp4 = win_pool.tile([WIN, 8 * WIN], BF16, tag="exp4")
                    nc.scalar.activation(
                        out=exp4, in_=sim4,
                        func=mybir.ActivationFunctionType.Exp,
                    )
                    own_view = bass.AP(
                        tensor=exp4.tensor, offset=exp4.offset,
                        ap=[list(exp4.ap[0]), [2 * WIN, 4], [1, WIN]],
                    )
                    nc.vector.tensor_mul(
                        own_view, own_view,
                        bass.AP(tensor=mask_sb.tensor, offset=mask_sb.offset,
                                ap=[list(mask_sb.ap[0]), [0, 4], [1, WIN]]),
                    )

                    # ---- PV + denominator for the 4 windows of the group
                    po = po_pool.tile([WIN, 4, D + 2], F32)
                    for j in range(4):
                        w = g0 + j
                        osl = po[:, j, 0 : D + 1]
                        own = exp4[:, 2 * WIN * j : 2 * WIN * j + WIN]
                        if w == 0:
                            nc.tensor.matmul(
                                osl, lhsT=own, rhs=vo_row[:, w, :],
                                start=True, stop=True,
                            )
                        else:
                            if j == 0:
                                prev = exp_prev[:, 7 * WIN : 8 * WIN]
                            else:
                                prev = exp4[:, 2 * WIN * j - WIN : 2 * WIN * j]
                            nc.tensor.matmul(
                                osl, lhsT=prev, rhs=vo_row[:, w - 1, :],
                                start=True, stop=False,
                            )
                            nc.tensor.matmul(
                                osl, lhsT=own, rhs=vo_row[:, w, :],
                                start=False, stop=True,
                            )

                    # ---- normalize: DVE reciprocal of the ones-column,
                    # then ACT copy-with-per-partition-scale per window
                    rec = rec_pool.tile([WIN, 4], F32, tag="rec")
                    den = bass.AP(tensor=po.tensor, offset=po.offset + D,
                                  ap=[list(po.ap[0]), [D + 2, 4]])
                    nc.vector.reciprocal(rec, den)
                    for j in range(4):
                        nc.scalar.activation(
                            out=out_row[:, g0 + j, :],
                            in_=po[:, j, 0:D],
                            func=mybir.ActivationFunctionType.Copy,
                            scale=rec[:, j : j + 1],
                        )

                nc.sync.dma_start(out=o_d[r], in_=out_row)

    nc.compile()
    return nc


_NC_CACHE = None


def _get_nc():
    global _NC_CACHE
    if _NC_CACHE is None:
        _NC_CACHE = build_bass()
    return _NC_CACHE


def _wmajor(a):
    # [ROWS, N, D] -> [ROWS, WIN, NW, D]: position-in-window major
    return np.ascontiguousarray(
        a.reshape(ROWS, NW, WIN, D).transpose(0, 2, 1, 3)
    )


def _bf(a):
    import ml_dtypes
    return np.ascontiguousarray(a).astype(ml_dtypes.bfloat16)


def _in_maps(q, k, v):
    q = _wmajor(np.asarray(q, dtype=np.float32).reshape(ROWS, N, D))
    k = _wmajor(np.asarray(k, dtype=np.float32).reshape(ROWS, N, D))
    v = _wmajor(np.asarray(v, dtype=np.float32).reshape(ROWS, N, D))
    vo = np.concatenate(
        [v, np.ones((ROWS, WIN, NW, 1), dtype=np.float32)], axis=-1)
    consts = _rot_consts()
    ident = np.eye(WIN, dtype=np.float32)
    # keep i >= jj  (rows jj = key position, cols i = query position)
    maskc = (np.arange(WIN)[None, :] >= np.arange(WIN)[:, None]).astype(
        np.float32)
    maps = []
    for cidx in range(NCORES):
        sl = slice(cidx * RPC, (cidx + 1) * RPC)
        m = {
            "q": _bf(q[sl]),
            "k": _bf(k[sl]),
            "vo": _bf(vo[sl]),
            "ident": _bf(ident),
            "maskc": _bf(maskc),
        }
        for name in CONST_NAMES:
            m[name] = _bf(consts[name])
        maps.append(m)
    return maps


def _run(q, k, v, **kw):
    nc = _get_nc()
    res = run_bass_kernel_spmd(nc, _in_maps(q, k, v), list(range(NCORES)), **kw)
    out = np.concatenate(
        [res.results[c]["o"].astype(np.float32) for c in range(NCORES)], axis=0)
    # [ROWS, WIN, NW, D] -> [ROWS, N, D]
    out = out.transpose(0, 2, 1, 3).reshape(B, H, N, D)
    return np.ascontiguousarray(out), res


def kernel(q, k, v):
    out, _ = _run(q, k, v)
    return out
